# revision 3
# baseline (speedup 1.0000x reference)
"""Causal self-attention (RoPE, 16 heads, d=64, B=4, T=2048, C=1024) on 8 TRN2 cores.

Sharding: core g = (batch b = g//2, head-group hg = g%2 covering 8 heads).
Data-parallel over B, tensor-parallel over heads.  Each core computes the
partial out-projection (its 8 heads' contribution, no bias); the host sums
the two head-group partials per batch and adds b_out.

Per-core kernel (all matmul operands bf16, fp32 PSUM accumulation):
  phase 1: qkv = xT.T @ Wqkv  (xT pre-transposed on host, stationary operand)
           RoPE on q,k in natural [t, d] layout (DVE, fp32 from PSUM)
           q,k DMA-transposed into [d, t] layout; v stored with a ones
           column appended per head (for the softmax denominator)
  phase 2: per 512-wide t-window, per head-pair: S^T[s,t] = k^T q matmuls
           (two heads packed in the PE array via tile_position row tiling,
           contraction dim is only 64), exp on ACT (scale=1/8, padding mask
           as per-partition bias, no max subtraction), causal upper blocks
           skipped, diagonal blocks masked with a triangular mask;
           AV+denominator accumulate in PSUM over s-blocks; normalize with
           per-partition reciprocal; attn DMA-transposed; out-projection.
"""

import os
from contextlib import ExitStack

import numpy as np
import ml_dtypes

B, T, C = 4, 2048, 1024
H, D = 16, 64
HG = 8            # heads per core
NCORES = 8
TB = T // 128     # 16 t-blocks of 128
CBN = C // 128    # 8 contraction chunks
NP = HG // 2      # 4 head pairs
NI = T // 512     # 4 t-windows of 512
ROPE_BASE = 10000.0

_PROG = None
_LAST_RESULTS = None


def _build_program():
    import concourse.bass as bass
    import concourse.tile as tile
    from concourse import bacc, mybir

    f32 = mybir.dt.float32
    bf = mybir.dt.bfloat16
    EXP = mybir.ActivationFunctionType.Exp

    nc = bacc.Bacc("TRN2", target_bir_lowering=False, debug=False)

    xT = nc.dram_tensor("xT", [C, T], bf, kind="ExternalInput").ap()
    wqkv = nc.dram_tensor("wqkv", [C, 3 * HG * D], bf, kind="ExternalInput").ap()
    wout = nc.dram_tensor("wout", [HG * D, C], bf, kind="ExternalInput").ap()
    cos8 = nc.dram_tensor("cos8", [T, HG * 32], f32, kind="ExternalInput").ap()
    sin8 = nc.dram_tensor("sin8", [T, HG * 32], f32, kind="ExternalInput").ap()
    padb = nc.dram_tensor("padb", [128, TB], f32, kind="ExternalInput").ap()
    dmask = nc.dram_tensor("dmask", [128, 128], bf, kind="ExternalInput").ap()
    outp = nc.dram_tensor("outp", [T, C], f32, kind="ExternalOutput").ap()

    with tile.TileContext(nc) as tc, ExitStack() as ctx:
        singles = ctx.enter_context(tc.tile_pool(name="singles", bufs=1))

        # ---- global SBUF tensors ----
        xt_sb = []
        for cb in range(CBN):
            t_ = singles.tile([128, T], bf, name=f"xt{cb}", tag=f"xt{cb}")
            nc.sync.dma_start(out=t_, in_=xT[cb * 128:(cb + 1) * 128, :])
            xt_sb.append(t_)
        w_sb = []
        for cb in range(CBN):
            t_ = singles.tile([128, 3 * HG * D], bf, name=f"w{cb}", tag=f"w{cb}")
            nc.sync.dma_start(out=t_, in_=wqkv[cb * 128:(cb + 1) * 128, :])
            w_sb.append(t_)
        wo_sb = []
        for c in range(4):
            t_ = singles.tile([128, C], bf, name=f"wo{c}", tag=f"wo{c}")
            nc.sync.dma_start(out=t_, in_=wout[c * 128:(c + 1) * 128, :])
            wo_sb.append(t_)
        cos_sb = singles.tile([128, TB, HG, 32], f32, name="cos_sb", tag="cos_sb")
        nc.sync.dma_start(
            out=cos_sb, in_=cos8.rearrange("(tb p) (h d) -> p tb h d", p=128, h=HG))
        sin_sb = singles.tile([128, TB, HG, 32], f32, name="sin_sb", tag="sin_sb")
        nc.sync.dma_start(
            out=sin_sb, in_=sin8.rearrange("(tb p) (h d) -> p tb h d", p=128, h=HG))
        padb_sb = singles.tile([128, TB], f32, name="padb_sb", tag="padb_sb")
        nc.sync.dma_start(out=padb_sb, in_=padb)
        dmask_sb = singles.tile([128, 128], bf, name="dmask_sb", tag="dmask_sb")
        nc.sync.dma_start(out=dmask_sb, in_=dmask)

        # q^T/k^T per head pair: rows 0-63 head 2p, rows 64-127 head 2p+1
        qT = [singles.tile([128, T], bf, name=f"qT{p}", tag=f"qT{p}") for p in range(NP)]
        kT = [singles.tile([128, T], bf, name=f"kT{p}", tag=f"kT{p}") for p in range(NP)]
        # v with a ones column per head: [s-block, head, 65]
        vones = singles.tile([128, TB, HG, D + 1], bf, name="vones", tag="vones")
        nc.vector.memset(vones[:, :, :, D:D + 1], 1.0)
        # zero operands for the PSUM-bank-clearing matmul (see phase 2)
        zeros = singles.tile([128, 512], bf, name="zeros", tag="zeros")
        nc.vector.memset(zeros, 0.0)

        # ---- phase 1: qkv projection + rope + transposes ----
        with tc.tile_pool(name="psum1", bufs=2, space="PSUM") as psum1, \
             tc.tile_pool(name="rope", bufs=8) as rope_pool, \
             tc.tile_pool(name="qknat", bufs=3) as qk_pool:
            for tb in range(TB):
                psq = psum1.tile([128, HG, D], f32, name="psq", tag="psq")
                psk = psum1.tile([128, HG, D], f32, name="psk", tag="psk")
                psv = psum1.tile([128, HG, D], f32, name="psv", tag="psv")
                for cb in range(CBN):
                    st = cb == 0
                    sp = cb == CBN - 1
                    lhs = xt_sb[cb][:, tb * 128:(tb + 1) * 128]
                    nc.tensor.matmul(psq, lhs, w_sb[cb][:, 0:512], start=st, stop=sp)
                    nc.tensor.matmul(psk, lhs, w_sb[cb][:, 512:1024], start=st, stop=sp)
                    nc.tensor.matmul(psv, lhs, w_sb[cb][:, 1024:1536], start=st, stop=sp)

                cos_t = cos_sb[:, tb]
                sin_t = sin_sb[:, tb]
                for ps, dst_name in ((psq, "q"), (psk, "k")):
                    x1 = ps[:, :, 0:32]
                    x2 = ps[:, :, 32:64]
                    t1 = rope_pool.tile([128, HG, 32], f32, name="t1", tag="rt")
                    t2 = rope_pool.tile([128, HG, 32], f32, name="t2", tag="rt")
                    t3 = rope_pool.tile([128, HG, 32], f32, name="t3", tag="rt")
                    t4 = rope_pool.tile([128, HG, 32], f32, name="t4", tag="rt")
                    nc.vector.tensor_mul(t1, x1, cos_t)
                    nc.vector.tensor_mul(t2, x2, sin_t)
                    nc.vector.tensor_mul(t3, x1, sin_t)
                    nc.vector.tensor_mul(t4, x2, cos_t)
                    ro = qk_pool.tile([128, HG, D], bf, name=f"{dst_name}ro", tag=f"{dst_name}ro")
                    nc.vector.tensor_sub(ro[:, :, 0:32], t1, t2)
                    nc.vector.tensor_add(ro[:, :, 32:64], t3, t4)
                    dstT = qT if dst_name == "q" else kT
                    for p in range(NP):
                        nc.sync.dma_start_transpose(
                            out=dstT[p][:, tb * 128:(tb + 1) * 128],
                            in_=ro[:, 2 * p:2 * p + 2, :])
                nc.vector.tensor_copy(out=vones[:, tb, :, 0:D], in_=psv)

        # ---- phase 2: attention + out-projection ----
        with tc.tile_pool(name="psum2", bufs=2, space="PSUM") as psum2, \
             tc.tile_pool(name="exps", bufs=3) as exp_pool, \
             tc.tile_pool(name="tris", bufs=4) as tri_pool, \
             tc.tile_pool(name="attn", bufs=2) as attn_pool, \
             tc.tile_pool(name="attnT", bufs=8) as attnT_pool, \
             tc.tile_pool(name="outsb", bufs=3) as out_pool, \
             tc.tile_pool(name="recips", bufs=8) as rc_pool:
            for I in range(NI):
                attn_i = attn_pool.tile([128, 4, HG, D], bf, name="attn_i", tag="attn_i")
                for p in range(NP):
                    psA = psum2.tile([128, 4, D + 1], f32, name="psA", tag="avA", bufs=1)
                    psB = psum2.tile([128, 4, D + 1], f32, name="psB", tag="avB", bufs=1)
                    # A start=True matmul clears has_written for the WHOLE psum
                    # bank, so interleaved accumulation groups in one bank need
                    # a single bank-wide zeroing matmul up front; every AV
                    # matmul then accumulates (per-element has_written bits).
                    for ps in (psA, psB):
                        nc.tensor.matmul(
                            ps[:, :, :], zeros[0:1, 0:128], zeros[0:1, 0:4 * (D + 1)],
                            start=True, stop=True, skip_group_check=True)

                    def emit_av(j, eA, eB):
                        jl = j - 4 * I
                        for il in range(max(jl, 0), 4):
                            for h2, (e, ps) in enumerate(((eA, psA), (eB, psB))):
                                nc.tensor.matmul(
                                    ps[:, il, :],
                                    e[:, il * 128:(il + 1) * 128],
                                    vones[:, j, 2 * p + h2, :],
                                    start=False, stop=(j == 4 * I + il),
                                    skip_group_check=True)

                    prev = None
                    for j in range(4 * I + 4):
                        jl = j - 4 * I
                        off = max(jl, 0) * 128
                        sA = psum2.tile([128, 512], f32, name="sA", tag="sA", bufs=2)
                        sB = psum2.tile([128, 512], f32, name="sB", tag="sB", bufs=2)
                        nc.tensor.matmul(
                            sA[:, off:512],
                            kT[p][0:64, j * 128:(j + 1) * 128],
                            qT[p][0:64, I * 512 + off:(I + 1) * 512],
                            start=True, stop=True, tile_position=(0, 0))
                        nc.tensor.matmul(
                            sB[:, off:512],
                            kT[p][64:128, j * 128:(j + 1) * 128],
                            qT[p][64:128, I * 512 + off:(I + 1) * 512],
                            start=True, stop=True, tile_position=(64, 0))
                        eA = exp_pool.tile([128, 512], bf, name="eA", tag="eA")
                        eB = exp_pool.tile([128, 512], bf, name="eB", tag="eB")
                        bias = padb_sb[:, j:j + 1]
                        for e, s_ in ((eA, sA), (eB, sB)):
                            if jl >= 0:
                                tri = tri_pool.tile([128, 128], bf, name="tri", tag="tri")
                                nc.scalar.activation(
                                    out=tri, in_=s_[:, off:off + 128],
                                    func=EXP, bias=bias, scale=0.125)
                                nc.vector.tensor_mul(e[:, off:off + 128], tri, dmask_sb)
                                if off + 128 < 512:
                                    nc.scalar.activation(
                                        out=e[:, off + 128:512], in_=s_[:, off + 128:512],
                                        func=EXP, bias=bias, scale=0.125)
                            else:
                                nc.scalar.activation(
                                    out=e, in_=s_, func=EXP, bias=bias, scale=0.125)
                        if prev is not None:
                            emit_av(*prev)
                        prev = (j, eA, eB)
                    emit_av(*prev)

                    for il in range(4):
                        for h2, ps in enumerate((psA, psB)):
                            rc = rc_pool.tile([128, 1], f32, name="rc", tag="rc")
                            nc.vector.reciprocal(rc, ps[:, il, D:D + 1])
                            nc.vector.tensor_scalar_mul(
                                attn_i[:, il, 2 * p + h2, :], ps[:, il, 0:D], rc)

                for il in range(4):
                    i = 4 * I + il
                    aT = attnT_pool.tile([128, 4, 128], bf, name="aT", tag="aT")
                    for c in range(4):
                        nc.sync.dma_start_transpose(
                            out=aT[:, c, :], in_=attn_i[:, il, 2 * c:2 * c + 2, :])
                    for n in range(2):
                        pso = psum2.tile([128, 512], f32, name="pso", tag="o", bufs=2)
                        for c in range(4):
                            nc.tensor.matmul(
                                pso, aT[:, c, :], wo_sb[c][:, n * 512:(n + 1) * 512],
                                start=(c == 0), stop=(c == 3))
                        osb = out_pool.tile([128, 512], f32, name="osb", tag="osb")
                        nc.vector.tensor_copy(out=osb, in_=pso)
                        nc.sync.dma_start(
                            out=outp[i * 128:(i + 1) * 128, n * 512:(n + 1) * 512],
                            in_=osb)

    nc.compile()
    return nc


def _get_program():
    global _PROG
    if _PROG is None:
        _PROG = _build_program()
    return _PROG


def _rope_tables():
    inv = 1.0 / (ROPE_BASE ** (np.arange(0, D, 2, dtype=np.float64) / D))
    f = np.arange(T, dtype=np.float64)[:, None] * inv[None, :]  # [T, 32]
    cos8 = np.tile(np.cos(f).astype(np.float32), (1, HG))       # [T, 256]
    sin8 = np.tile(np.sin(f).astype(np.float32), (1, HG))
    return cos8, sin8


def kernel(x, attention_mask, W_qkv, W_out, b_out):
    global _LAST_RESULTS
    from concourse.bass_utils import run_bass_kernel_spmd

    nc = _get_program()
    bf16 = ml_dtypes.bfloat16
    x = np.asarray(x, dtype=np.float32)
    attention_mask = np.asarray(attention_mask)
    W_qkv = np.asarray(W_qkv, dtype=np.float32)
    W_out = np.asarray(W_out, dtype=np.float32)
    b_out = np.asarray(b_out, dtype=np.float32)

    cos8, sin8 = _rope_tables()
    dmask = (np.arange(128)[:, None] <= np.arange(128)[None, :]).astype(bf16)

    in_maps = []
    for g in range(NCORES):
        b, hg = g // 2, g % 2
        sl = slice(hg * 512, hg * 512 + 512)
        wq = W_qkv[:, 0 * C:][:, sl]
        wk = W_qkv[:, 1 * C:2 * C][:, sl]
        wv = W_qkv[:, 2 * C:3 * C][:, sl]
        wqkv_g = np.ascontiguousarray(
            np.concatenate([wq, wk, wv], axis=1)).astype(bf16)
        xT_g = np.ascontiguousarray(x[b].T).astype(bf16)
        wout_g = np.ascontiguousarray(W_out[sl, :]).astype(bf16)
        padb_g = np.ascontiguousarray(
            np.where(attention_mask[b] != 0, 0.0, -1e30)
            .astype(np.float32).reshape(TB, 128).T)
        in_maps.append({
            "xT": xT_g, "wqkv": wqkv_g, "wout": wout_g,
            "cos8": cos8, "sin8": sin8, "padb": padb_g, "dmask": dmask,
        })

    res = run_bass_kernel_spmd(nc, in_maps, list(range(NCORES)))
    _LAST_RESULTS = res
    out = np.empty((B, T, C), dtype=np.float32)
    for b in range(B):
        out[b] = res.results[2 * b]["outp"] + res.results[2 * b + 1]["outp"] + b_out
    return out


# revision 9
# speedup vs baseline: 1.0994x; 1.0994x over previous
"""Causal self-attention (RoPE, 16 heads, d=64, B=4, T=2048, C=1024) on 8 TRN2 cores.

Sharding: core g = (batch b = g//2, head-group hg = g%2 covering 8 heads).
Data-parallel over B, tensor-parallel over heads.  Each core computes the
partial out-projection (its 8 heads' contribution, no bias); the host sums
the two head-group partials per batch and adds b_out.

Per-core kernel (all matmul operands bf16, fp32 PSUM accumulation):
  phase 1: qkv = xT.T @ Wqkv  (xT pre-transposed on host, stationary operand)
           RoPE on q,k in natural [t, d] layout (DVE, fp32 from PSUM)
           q,k HW-DMA-transposed into [d, t] layout; v copied with a ones
           column appended per head (denominator comes out of the AV matmul)
  phase 2: per 512-wide t-window, per head-pair:
           S^T[s,t] = k^T q (two heads packed in the PE array via
           tile_position row tiling since the contraction dim is 64);
           exp on ACT (scale=1/8, padding mask as per-partition bias, no max
           subtraction -- logits are ~N(0,1)); causal upper blocks skipped,
           diagonal blocks masked with a triangular mask; per head,
           [V | 1] is the stationary operand and exp(S^T) streams:
           psT[0:64] accumulates attn_out^T[d,t], psT[64] the denominator.
           Normalizer broadcast to 128 partitions via a K=1 PE matmul;
           attn^T normalized on DVE straight into the out-projection's
           stationary layout -- no attention-output transpose needed.
"""

import os
from contextlib import ExitStack

import numpy as np
import ml_dtypes

B, T, C = 4, 2048, 1024
H, D = 16, 64
HG = 8            # heads per core
NCORES = 8
TB = T // 128     # 16 t/s-blocks of 128
CBN = C // 128    # 8 contraction chunks
NP = HG // 2      # 4 head pairs
NI = T // 512     # 4 t-windows of 512
ROPE_BASE = 10000.0

_PROG = None
_LAST_RESULTS = None


def _build_program():
    import concourse.bass as bass
    import concourse.tile as tile
    from concourse import bacc, mybir

    f32 = mybir.dt.float32
    bf = mybir.dt.bfloat16
    EXP = mybir.ActivationFunctionType.Exp

    nc = bacc.Bacc("TRN2", target_bir_lowering=False, debug=False)

    xT = nc.dram_tensor("xT", [C, T], bf, kind="ExternalInput").ap()
    wqkv = nc.dram_tensor("wqkv", [C, 3 * HG * D], bf, kind="ExternalInput").ap()
    wout = nc.dram_tensor("wout", [HG * D, C], bf, kind="ExternalInput").ap()
    cos8 = nc.dram_tensor("cos8", [T, HG * 32], f32, kind="ExternalInput").ap()
    sin8 = nc.dram_tensor("sin8", [T, HG * 32], f32, kind="ExternalInput").ap()
    padb = nc.dram_tensor("padb", [128, TB], f32, kind="ExternalInput").ap()
    dmask = nc.dram_tensor("dmask", [128, 256], bf, kind="ExternalInput").ap()
    outp = nc.dram_tensor("outp", [T, C], f32, kind="ExternalOutput").ap()

    with tile.TileContext(nc) as tc, ExitStack() as ctx:
        singles = ctx.enter_context(tc.tile_pool(name="singles", bufs=1))

        # ---- global SBUF tensors (input loads on the GpSimd SWDGE queue;
        # the Sync queue is reserved for DMA transposes so the xbar never
        # flips modes) ----
        xt_sb = []
        for cb in range(CBN):
            t_ = singles.tile([128, T], bf, name=f"xt{cb}", tag=f"xt{cb}")
            nc.gpsimd.dma_start(out=t_, in_=xT[cb * 128:(cb + 1) * 128, :])
            xt_sb.append(t_)
        w_sb = []
        for cb in range(CBN):
            t_ = singles.tile([128, 3 * HG * D], bf, name=f"w{cb}", tag=f"w{cb}")
            nc.gpsimd.dma_start(out=t_, in_=wqkv[cb * 128:(cb + 1) * 128, :])
            w_sb.append(t_)
        wo_sb = []
        for c in range(4):
            t_ = singles.tile([128, C], bf, name=f"wo{c}", tag=f"wo{c}")
            nc.gpsimd.dma_start(out=t_, in_=wout[c * 128:(c + 1) * 128, :])
            wo_sb.append(t_)
        cos_sb = singles.tile([128, TB, HG, 32], f32, name="cos_sb", tag="cos_sb")
        nc.gpsimd.dma_start(
            out=cos_sb, in_=cos8.rearrange("(tb p) (h d) -> p tb h d", p=128, h=HG))
        sin_sb = singles.tile([128, TB, HG, 32], f32, name="sin_sb", tag="sin_sb")
        nc.gpsimd.dma_start(
            out=sin_sb, in_=sin8.rearrange("(tb p) (h d) -> p tb h d", p=128, h=HG))
        padb_sb = singles.tile([128, TB], f32, name="padb_sb", tag="padb_sb")
        nc.gpsimd.dma_start(out=padb_sb, in_=padb)
        dmask_sb = singles.tile([128, 2, 128], bf, name="dmask_sb", tag="dmask_sb")
        nc.gpsimd.dma_start(out=dmask_sb, in_=dmask)

        # q^T/k^T: [within-pair col (head-lo d / head-hi d), s-block, pair, t]
        qT_all = singles.tile([128, TB, NP, 128], bf, name="qT_all", tag="qT_all")
        kT_all = singles.tile([128, TB, NP, 128], bf, name="kT_all", tag="kT_all")
        # v with a ones column per head: [s-block, head, 65]
        vones = singles.tile([128, TB, HG, D + 1], bf, name="vones", tag="vones")
        nc.vector.memset(vones[:, :, :, D:D + 1], 1.0)
        ones_row = singles.tile([1, 128], bf, name="ones_row", tag="ones_row")
        nc.vector.memset(ones_row, 1.0)

        # ---- phase 1: qkv projection + rope + transposes ----
        with tc.tile_pool(name="psum1", bufs=2, space="PSUM") as psum1, \
             tc.tile_pool(name="rope", bufs=8) as rope_pool, \
             tc.tile_pool(name="qknat", bufs=3) as qk_pool:
            for tb in range(TB):
                psq = psum1.tile([128, HG, D], f32, name="psq", tag="psq")
                psk = psum1.tile([128, HG, D], f32, name="psk", tag="psk")
                psv = psum1.tile([128, HG, D], f32, name="psv", tag="psv")
                for cb in range(CBN):
                    st = cb == 0
                    sp = cb == CBN - 1
                    lhs = xt_sb[cb][:, tb * 128:(tb + 1) * 128]
                    nc.tensor.matmul(psq, lhs, w_sb[cb][:, 0:512], start=st, stop=sp)
                    nc.tensor.matmul(psk, lhs, w_sb[cb][:, 512:1024], start=st, stop=sp)
                    nc.tensor.matmul(psv, lhs, w_sb[cb][:, 1024:1536], start=st, stop=sp)

                cos_t = cos_sb[:, tb]
                sin_t = sin_sb[:, tb]
                for ps, dst in ((psq, qT_all), (psk, kT_all)):
                    x1 = ps[:, :, 0:32]
                    x2 = ps[:, :, 32:64]
                    t1 = rope_pool.tile([128, HG, 32], f32, name="t1", tag="rt")
                    t2 = rope_pool.tile([128, HG, 32], f32, name="t2", tag="rt")
                    t3 = rope_pool.tile([128, HG, 32], f32, name="t3", tag="rt")
                    t4 = rope_pool.tile([128, HG, 32], f32, name="t4", tag="rt")
                    nc.vector.tensor_mul(t1, x1, cos_t)
                    nc.vector.tensor_mul(t2, x2, sin_t)
                    nc.vector.tensor_mul(t3, x1, sin_t)
                    nc.vector.tensor_mul(t4, x2, cos_t)
                    ro = qk_pool.tile([128, HG, D], bf, name="ro", tag="ro")
                    nc.vector.tensor_sub(ro[:, :, 0:32], t1, t2)
                    nc.vector.tensor_add(ro[:, :, 32:64], t3, t4)
                    nc.sync.dma_start_transpose(out=dst[:, tb, :, :], in_=ro)
                nc.vector.tensor_copy(out=vones[:, tb, :, 0:D], in_=psv)

        # ---- phase 2: attention + out-projection ----
        with tc.tile_pool(name="psum2", bufs=2, space="PSUM") as psum2, \
             tc.tile_pool(name="exps", bufs=3) as exp_pool, \
             tc.tile_pool(name="tris", bufs=4) as tri_pool, \
             tc.tile_pool(name="attnT", bufs=2) as aT_pool, \
             tc.tile_pool(name="rrows", bufs=4) as rr_pool, \
             tc.tile_pool(name="outsb", bufs=3) as out_pool:
            for I in range(NI):
                aT_I = aT_pool.tile([128, NP, 512], bf, name="aT_I", tag="aT_I")
                for p in range(NP):
                    psTA = psum2.tile([D + 1, 512], f32, name="psTA", tag="avA", bufs=1)
                    psTB = psum2.tile([D + 1, 512], f32, name="psTB", tag="avB", bufs=1)

                    def emit_av(j, eAB):
                        off = max(j - 4 * I, 0) * 128
                        for h2, psT in ((0, psTA), (1, psTB)):
                            nc.tensor.matmul(
                                psT[:, off:512],
                                vones[:, j, 2 * p + h2, :],
                                eAB[:, h2, off:512],
                                start=(j == 0), stop=(j == 4 * I + 3))

                    prev = None
                    for j in range(4 * I + 4):
                        jl = j - 4 * I
                        off = max(jl, 0) * 128
                        sAB = psum2.tile([128, 2, 512], f32, name="sAB", tag="sAB", bufs=2)
                        nc.tensor.matmul(
                            sAB[:, 0, off:512],
                            kT_all[0:64, j, p, :],
                            qT_all[0:64, 4 * I + max(jl, 0):4 * I + 4, p, :],
                            start=True, stop=True, tile_position=(0, 0))
                        nc.tensor.matmul(
                            sAB[:, 1, off:512],
                            kT_all[64:128, j, p, :],
                            qT_all[64:128, 4 * I + max(jl, 0):4 * I + 4, p, :],
                            start=True, stop=True, tile_position=(64, 0))
                        eAB = exp_pool.tile([128, 2, 512], bf, name="eAB", tag="eAB")
                        bias = padb_sb[:, j:j + 1]
                        if jl >= 0:
                            tri = tri_pool.tile([128, 2, 128], bf, name="tri", tag="tri")
                            nc.scalar.activation(
                                out=tri, in_=sAB[:, :, off:off + 128],
                                func=EXP, bias=bias, scale=0.125)
                            nc.vector.tensor_mul(eAB[:, :, off:off + 128], tri, dmask_sb)
                            if off + 128 < 512:
                                nc.scalar.activation(
                                    out=eAB[:, :, off + 128:512],
                                    in_=sAB[:, :, off + 128:512],
                                    func=EXP, bias=bias, scale=0.125)
                        else:
                            nc.scalar.activation(
                                out=eAB, in_=sAB, func=EXP, bias=bias, scale=0.125)
                        if prev is not None:
                            emit_av(*prev)
                        prev = (j, eAB)
                    emit_av(*prev)

                    # normalize + evacuate transposed attn straight into the
                    # out-projection's stationary layout; the reciprocal row is
                    # partition-broadcast to 64 rows via a SBUF->SBUF DMA so the
                    # multiply reads only one PSUM operand
                    for h2, psT in ((0, psTA), (1, psTB)):
                        rr = rr_pool.tile([1, 512], bf, name="rr", tag="rr")
                        with nc.allow_low_precision(reason="bf16 softmax normalizer"):
                            nc.vector.reciprocal(rr, psT[D:D + 1, :])
                        rb = psum2.tile([128, 512], f32, name="rb", tag="work", bufs=2)
                        nc.tensor.matmul(rb, ones_row, rr, start=True, stop=True)
                        evac = rr_pool.tile([64, 512], bf, name="evac", tag="evac")
                        nc.vector.tensor_copy(out=evac, in_=psT[0:D, :])
                        nc.vector.tensor_mul(
                            aT_I[h2 * 64:(h2 + 1) * 64, p, :], evac, rb[0:64, :])

                for il in range(4):
                    i = 4 * I + il
                    for n in range(2):
                        pso = psum2.tile([128, 512], f32, name="pso", tag="work", bufs=2)
                        for c in range(4):
                            nc.tensor.matmul(
                                pso,
                                aT_I[:, c, il * 128:(il + 1) * 128],
                                wo_sb[c][:, n * 512:(n + 1) * 512],
                                start=(c == 0), stop=(c == 3))
                        osb = out_pool.tile([128, 512], f32, name="osb", tag="osb")
                        nc.vector.tensor_copy(out=osb, in_=pso)
                        nc.gpsimd.dma_start(
                            out=outp[i * 128:(i + 1) * 128, n * 512:(n + 1) * 512],
                            in_=osb)

    nc.compile()
    return nc


def _get_program():
    global _PROG
    if _PROG is None:
        _PROG = _build_program()
    return _PROG


def _rope_tables():
    inv = 1.0 / (ROPE_BASE ** (np.arange(0, D, 2, dtype=np.float64) / D))
    f = np.arange(T, dtype=np.float64)[:, None] * inv[None, :]  # [T, 32]
    cos8 = np.tile(np.cos(f).astype(np.float32), (1, HG))       # [T, 256]
    sin8 = np.tile(np.sin(f).astype(np.float32), (1, HG))
    return cos8, sin8


def kernel(x, attention_mask, W_qkv, W_out, b_out):
    global _LAST_RESULTS
    from concourse.bass_utils import run_bass_kernel_spmd

    nc = _get_program()
    bf16 = ml_dtypes.bfloat16
    x = np.asarray(x, dtype=np.float32)
    attention_mask = np.asarray(attention_mask)
    W_qkv = np.asarray(W_qkv, dtype=np.float32)
    W_out = np.asarray(W_out, dtype=np.float32)
    b_out = np.asarray(b_out, dtype=np.float32)

    cos8, sin8 = _rope_tables()
    tri = (np.arange(128)[:, None] <= np.arange(128)[None, :])
    dmask = np.concatenate([tri, tri], axis=1).astype(bf16)

    in_maps = []
    for g in range(NCORES):
        b, hg = g // 2, g % 2
        sl = slice(hg * 512, hg * 512 + 512)
        wq = W_qkv[:, 0 * C:][:, sl]
        wk = W_qkv[:, 1 * C:2 * C][:, sl]
        wv = W_qkv[:, 2 * C:3 * C][:, sl]
        wqkv_g = np.ascontiguousarray(
            np.concatenate([wq, wk, wv], axis=1)).astype(bf16)
        xT_g = np.ascontiguousarray(x[b].T).astype(bf16)
        wout_g = np.ascontiguousarray(W_out[sl, :]).astype(bf16)
        padb_g = np.ascontiguousarray(
            np.where(attention_mask[b] != 0, 0.0, -1e30)
            .astype(np.float32).reshape(TB, 128).T)
        in_maps.append({
            "xT": xT_g, "wqkv": wqkv_g, "wout": wout_g,
            "cos8": cos8, "sin8": sin8, "padb": padb_g, "dmask": dmask,
        })

    res = run_bass_kernel_spmd(nc, in_maps, list(range(NCORES)))
    _LAST_RESULTS = res
    out = np.empty((B, T, C), dtype=np.float32)
    for b in range(B):
        out[b] = res.results[2 * b]["outp"] + res.results[2 * b + 1]["outp"] + b_out
    return out


# revision 13
# speedup vs baseline: 1.1303x; 1.0281x over previous
"""Causal self-attention (RoPE, 16 heads, d=64, B=4, T=2048, C=1024) on 8 TRN2 cores.

Sharding: core g = (batch b = g//2, head-group hg = g%2 covering 8 heads).
Data-parallel over B, tensor-parallel over heads.  Each core computes the
partial out-projection (its 8 heads' contribution, no bias); the host sums
the two head-group partials per batch and adds b_out.

Per-core kernel (all matmul operands bf16, fp32 PSUM accumulation):
  phase 1: qkv = xT.T @ Wqkv  (xT pre-transposed on host, stationary operand)
           RoPE on q,k in natural [t, d] layout (DVE, fp32 from PSUM)
           q,k HW-DMA-transposed into [d, t] layout; v copied with a ones
           column appended per head (denominator comes out of the AV matmul)
  phase 2: per 512-wide t-window, per head-pair:
           S^T[s,t] = k^T q (two heads packed in the PE array via
           tile_position row tiling since the contraction dim is 64);
           exp on ACT (scale=1/8, padding mask as per-partition bias, no max
           subtraction -- logits are ~N(0,1)); causal upper blocks skipped,
           diagonal blocks masked with a triangular mask; per head,
           [V | 1] is the stationary operand and exp(S^T) streams:
           psT[0:64] accumulates attn_out^T[d,t], psT[64] the denominator.
           Normalizer broadcast to 128 partitions via a K=1 PE matmul;
           attn^T normalized on DVE straight into the out-projection's
           stationary layout -- no attention-output transpose needed.
"""

import os
from contextlib import ExitStack

import numpy as np
import ml_dtypes

B, T, C = 4, 2048, 1024
H, D = 16, 64
HG = 8            # heads per core
NCORES = 8
TB = T // 128     # 16 t/s-blocks of 128
CBN = C // 128    # 8 contraction chunks
NP = HG // 2      # 4 head pairs
NI = T // 512     # 4 t-windows of 512
ROPE_BASE = 10000.0

_PROG = None
_LAST_RESULTS = None


def _build_program():
    import concourse.bass as bass
    import concourse.tile as tile
    from concourse import bacc, mybir

    f32 = mybir.dt.float32
    bf = mybir.dt.bfloat16
    EXP = mybir.ActivationFunctionType.Exp

    nc = bacc.Bacc("TRN2", target_bir_lowering=False, debug=False)

    xT = nc.dram_tensor("xT", [C, T], bf, kind="ExternalInput").ap()
    wqkv = nc.dram_tensor("wqkv", [C, 3 * HG * D], bf, kind="ExternalInput").ap()
    wout = nc.dram_tensor("wout", [HG * D, C], bf, kind="ExternalInput").ap()
    cos8 = nc.dram_tensor("cos8", [T, HG * D], bf, kind="ExternalInput").ap()
    sin8 = nc.dram_tensor("sin8", [T, HG * D], bf, kind="ExternalInput").ap()
    padb = nc.dram_tensor("padb", [128, TB], f32, kind="ExternalInput").ap()
    dmask = nc.dram_tensor("dmask", [128, 256], bf, kind="ExternalInput").ap()
    outp = nc.dram_tensor("outp", [T, C], f32, kind="ExternalOutput").ap()

    with tile.TileContext(nc) as tc, ExitStack() as ctx:
        singles = ctx.enter_context(tc.tile_pool(name="singles", bufs=1))

        # ---- global SBUF tensors (input loads on the GpSimd SWDGE queue;
        # the Sync queue is reserved for DMA transposes so the xbar never
        # flips modes) ----
        xt_sb = []
        for cb in range(CBN):
            t_ = singles.tile([128, T], bf, name=f"xt{cb}", tag=f"xt{cb}")
            nc.gpsimd.dma_start(out=t_, in_=xT[cb * 128:(cb + 1) * 128, :])
            xt_sb.append(t_)
        w_sb = []
        for cb in range(CBN):
            t_ = singles.tile([128, 3 * HG * D], bf, name=f"w{cb}", tag=f"w{cb}")
            nc.gpsimd.dma_start(out=t_, in_=wqkv[cb * 128:(cb + 1) * 128, :])
            w_sb.append(t_)
        wo_sb = []
        for c in range(4):
            t_ = singles.tile([128, C], bf, name=f"wo{c}", tag=f"wo{c}")
            nc.gpsimd.dma_start(out=t_, in_=wout[c * 128:(c + 1) * 128, :])
            wo_sb.append(t_)
        cos_sb = singles.tile([128, TB, HG, D], bf, name="cos_sb", tag="cos_sb")
        nc.gpsimd.dma_start(
            out=cos_sb, in_=cos8.rearrange("(tb p) (h d) -> p tb h d", p=128, h=HG))
        sin_sb = singles.tile([128, TB, HG, D], bf, name="sin_sb", tag="sin_sb")
        nc.gpsimd.dma_start(
            out=sin_sb, in_=sin8.rearrange("(tb p) (h d) -> p tb h d", p=128, h=HG))
        padb_sb = singles.tile([128, TB], f32, name="padb_sb", tag="padb_sb")
        nc.gpsimd.dma_start(out=padb_sb, in_=padb)
        dmask_sb = singles.tile([128, 2, 128], bf, name="dmask_sb", tag="dmask_sb")
        nc.gpsimd.dma_start(out=dmask_sb, in_=dmask)

        # q^T/k^T: [within-pair col (head-lo d / head-hi d), s-block, pair, t]
        qT_all = singles.tile([128, TB, NP, 128], bf, name="qT_all", tag="qT_all")
        kT_all = singles.tile([128, TB, NP, 128], bf, name="kT_all", tag="kT_all")
        # v with 64 ones columns per head: the AV matmul then emits the
        # softmax denominator replicated on 64 partitions (rows 64-127),
        # so the reciprocal is a wide DVE op instead of a 1-partition crawl
        vones = singles.tile([128, TB, HG, 128], bf, name="vones", tag="vones")
        nc.vector.memset(vones[:, :, :, D:128], 1.0)

        # qkv + attention interleaved per 512-wide t-window so the PE stream
        # stays dense (HAM stays at 2.4 GHz): attention for window I only
        # needs q/k/v blocks 0..4I+3, which segment I of the qkv loop topped
        # off.  One shared PSUM pool: qkv 2 + sAB 2x2 + psT 2 = 8 banks.
        with tc.tile_pool(name="psum", bufs=2, space="PSUM") as psum, \
             tc.tile_pool(name="rope", bufs=4) as rope_pool, \
             tc.tile_pool(name="qknat", bufs=3) as qk_pool, \
             tc.tile_pool(name="exps", bufs=3) as exp_pool, \
             tc.tile_pool(name="tris", bufs=2) as tri_pool, \
             tc.tile_pool(name="attnT", bufs=2) as aT_pool, \
             tc.tile_pool(name="recips", bufs=2) as rc_pool, \
             tc.tile_pool(name="outsb", bufs=2) as out_pool:
            for I in range(NI):
                # ---- qkv segment: t-blocks 4I..4I+3 ----
                for tb in range(4 * I, 4 * I + 4):
                    for which, base in (("q", 0), ("k", 512), ("v", 1024)):
                        ps = psum.tile([128, HG, D], f32, name=f"ps{which}", tag="qkv")
                        for cb in range(CBN):
                            nc.tensor.matmul(
                                ps, xt_sb[cb][:, tb * 128:(tb + 1) * 128],
                                w_sb[cb][:, base:base + 512],
                                start=(cb == 0), stop=(cb == CBN - 1))
                        if which == "v":
                            nc.vector.tensor_copy(out=vones[:, tb, :, 0:D], in_=ps)
                            continue
                        # rope: P_c = qkv*cosF, P_s = qkv*sinF (tables hold the
                        # cos/sin value for BOTH halves of each head), then
                        # lo = P_c.lo - P_s.hi ; hi = P_s.lo + P_c.hi
                        pc = rope_pool.tile([128, HG, D], f32, name="pc", tag="rt")
                        psn = rope_pool.tile([128, HG, D], f32, name="psn", tag="rt")
                        nc.vector.tensor_mul(pc, ps, cos_sb[:, tb])
                        nc.vector.tensor_mul(psn, ps, sin_sb[:, tb])
                        ro = qk_pool.tile([128, HG, D], bf, name="ro", tag="ro")
                        nc.vector.tensor_sub(
                            ro[:, :, 0:32], pc[:, :, 0:32], psn[:, :, 32:64])
                        nc.vector.tensor_add(
                            ro[:, :, 32:64], psn[:, :, 0:32], pc[:, :, 32:64])
                        dst = qT_all if which == "q" else kT_all
                        nc.sync.dma_start_transpose(out=dst[:, tb, :, :], in_=ro)

                # ---- attention window I ----
                aT_I = aT_pool.tile([128, NP, 512], bf, name="aT_I", tag="aT_I")
                for p in range(NP):
                    psTA = psum.tile([128, 512], f32, name="psTA", tag="avA", bufs=1)
                    psTB = psum.tile([128, 512], f32, name="psTB", tag="avB", bufs=1)

                    def emit_av(j, eAB):
                        off = max(j - 4 * I, 0) * 128
                        for h2, psT in ((0, psTA), (1, psTB)):
                            nc.tensor.matmul(
                                psT[:, off:512],
                                vones[:, j, 2 * p + h2, :],
                                eAB[:, h2, off:512],
                                start=(j == 0), stop=(j == 4 * I + 3))

                    prev = None
                    for j in range(4 * I + 4):
                        jl = j - 4 * I
                        off = max(jl, 0) * 128
                        sAB = psum.tile([128, 2, 512], f32, name="sAB", tag="sAB", bufs=2)
                        nc.tensor.matmul(
                            sAB[:, 0, off:512],
                            kT_all[0:64, j, p, :],
                            qT_all[0:64, 4 * I + max(jl, 0):4 * I + 4, p, :],
                            start=True, stop=True, tile_position=(0, 0))
                        nc.tensor.matmul(
                            sAB[:, 1, off:512],
                            kT_all[64:128, j, p, :],
                            qT_all[64:128, 4 * I + max(jl, 0):4 * I + 4, p, :],
                            start=True, stop=True, tile_position=(64, 0))
                        eAB = exp_pool.tile([128, 2, 512], bf, name="eAB", tag="eAB")
                        bias = padb_sb[:, j:j + 1]
                        if jl >= 0:
                            tri = tri_pool.tile([128, 2, 128], bf, name="tri", tag="tri")
                            nc.scalar.activation(
                                out=tri, in_=sAB[:, :, off:off + 128],
                                func=EXP, bias=bias, scale=0.125)
                            nc.vector.tensor_mul(eAB[:, :, off:off + 128], tri, dmask_sb)
                            if off + 128 < 512:
                                nc.scalar.activation(
                                    out=eAB[:, :, off + 128:512],
                                    in_=sAB[:, :, off + 128:512],
                                    func=EXP, bias=bias, scale=0.125)
                        else:
                            nc.scalar.activation(
                                out=eAB, in_=sAB, func=EXP, bias=bias, scale=0.125)
                        if prev is not None:
                            emit_av(*prev)
                        prev = (j, eAB)
                    emit_av(*prev)

                    # normalize + evacuate transposed attn straight into the
                    # out-projection's stationary layout
                    for h2, psT in ((0, psTA), (1, psTB)):
                        rc = rc_pool.tile([64, 512], f32, name="rc", tag="rc")
                        nc.vector.reciprocal(rc, psT[D:128, :])
                        nc.vector.tensor_mul(
                            aT_I[h2 * 64:(h2 + 1) * 64, p, :], psT[0:D, :], rc)

                for il in range(4):
                    i = 4 * I + il
                    for n in range(2):
                        pso = psum.tile([128, 512], f32, name="pso", tag="sAB", bufs=2)
                        for c in range(4):
                            nc.tensor.matmul(
                                pso,
                                aT_I[:, c, il * 128:(il + 1) * 128],
                                wo_sb[c][:, n * 512:(n + 1) * 512],
                                start=(c == 0), stop=(c == 3))
                        osb = out_pool.tile([128, 512], f32, name="osb", tag="osb")
                        nc.vector.tensor_copy(out=osb, in_=pso)
                        nc.gpsimd.dma_start(
                            out=outp[i * 128:(i + 1) * 128, n * 512:(n + 1) * 512],
                            in_=osb)

    nc.compile()
    return nc


def _get_program():
    global _PROG
    if _PROG is None:
        _PROG = _build_program()
    return _PROG


def _rope_tables():
    bf16 = ml_dtypes.bfloat16
    inv = 1.0 / (ROPE_BASE ** (np.arange(0, D, 2, dtype=np.float64) / D))
    f = np.arange(T, dtype=np.float64)[:, None] * inv[None, :]  # [T, 32]
    c = np.cos(f)
    s = np.sin(f)
    # per head, both 32-col halves carry the same table value
    cos8 = np.tile(np.concatenate([c, c], axis=1), (1, HG)).astype(bf16)  # [T, 512]
    sin8 = np.tile(np.concatenate([s, s], axis=1), (1, HG)).astype(bf16)
    return cos8, sin8


def kernel(x, attention_mask, W_qkv, W_out, b_out):
    global _LAST_RESULTS
    from concourse.bass_utils import run_bass_kernel_spmd

    nc = _get_program()
    bf16 = ml_dtypes.bfloat16
    x = np.asarray(x, dtype=np.float32)
    attention_mask = np.asarray(attention_mask)
    W_qkv = np.asarray(W_qkv, dtype=np.float32)
    W_out = np.asarray(W_out, dtype=np.float32)
    b_out = np.asarray(b_out, dtype=np.float32)

    cos8, sin8 = _rope_tables()
    tri = (np.arange(128)[:, None] <= np.arange(128)[None, :])
    dmask = np.concatenate([tri, tri], axis=1).astype(bf16)

    in_maps = []
    for g in range(NCORES):
        b, hg = g // 2, g % 2
        sl = slice(hg * 512, hg * 512 + 512)
        wq = W_qkv[:, 0 * C:][:, sl]
        wk = W_qkv[:, 1 * C:2 * C][:, sl]
        wv = W_qkv[:, 2 * C:3 * C][:, sl]
        wqkv_g = np.ascontiguousarray(
            np.concatenate([wq, wk, wv], axis=1)).astype(bf16)
        xT_g = np.ascontiguousarray(x[b].T).astype(bf16)
        wout_g = np.ascontiguousarray(W_out[sl, :]).astype(bf16)
        padb_g = np.ascontiguousarray(
            np.where(attention_mask[b] != 0, 0.0, -1e30)
            .astype(np.float32).reshape(TB, 128).T)
        in_maps.append({
            "xT": xT_g, "wqkv": wqkv_g, "wout": wout_g,
            "cos8": cos8, "sin8": sin8, "padb": padb_g, "dmask": dmask,
        })

    res = run_bass_kernel_spmd(nc, in_maps, list(range(NCORES)))
    _LAST_RESULTS = res
    out = np.empty((B, T, C), dtype=np.float32)
    for b in range(B):
        out[b] = res.results[2 * b]["outp"] + res.results[2 * b + 1]["outp"] + b_out
    return out
